# revision 18
# baseline (speedup 1.0000x reference)
"""Trainium2 Bass kernel for the GRAND attention block.

Shapes (hardcoded): B=16, C=1024, F=512, H=8, D=128, HD=1024.
Sharding: batch dim split across 8 cores (2 batches per core), weights
replicated; no collectives needed.

Math per batch (b):
  P_q = (x Wq + bq)/sqrt(D), P_k = x Wk + bk, P_v = x Wv + bv  [1024, 1024]
  The reference reshape [C, H*D] -> [H, C, D] (no permute) makes
  "head" g = proj_row // 128 and attention row c'' = 128*e + m where
  e = colblock, m = proj_row % 128.  Attention runs independently inside
  each group g of 1024 rows.

    S^T tiles = K_e2 Q_e1^T  -> exp -> Z = E^T            [1024, 1024]
    r = column sums of Z (DVE tree + accumulating ones-matmuls)
    Z' = Z - diag(r)   (handles softmax denominator and the "- I" term)
    vals^T = (V_g^T Z') / r
    out^T += W0_g^T vals^T  (PSUM-accumulated over g), + bw0
  out^T is DMA-xbar transposed and stored with the unscrambling view.

Perf design: all dtype casts and weight/x layout transforms happen on the
HOST (numpy) so the device sees bf16/fp8 tensors it can DMA directly —
the fp32 load path was DMA-bandwidth-bound (~80us startup).  Q/K
projections run as fp8 DoubleRow matmuls (K folded 512 -> 2x256, weights
host-prescaled by 64 to dodge e4m3 subnormals; 1/64 and 1/sqrt(D) fold
into the drain's activation scale).  Attention is software-pipelined:
scores+exp of group g are emitted before the tail of g-1 so the ACT exp
stream never starves; column sums use a DVE half-reduction plus
accumulating ones-matmuls; the diag subtraction runs on GpSimd; vals are
scaled by reciprocal_approx_fast.  PSUM: 2x two-bank score tiles, 2x
one-bank r/vals tiles, one two-bank out accumulator.
"""

import math

import numpy as np
import ml_dtypes

import concourse.bass as bass
import concourse.bacc as bacc
import concourse.mybir as mybir
import concourse.tile as tile
from concourse.masks import make_identity
from concourse.bass_utils import run_bass_kernel_spmd

F32 = mybir.dt.float32
BF16 = mybir.dt.bfloat16
FP8 = mybir.dt.float8e4

N_CORES = 8
B_PER = 2  # batches per core
C = 1024
F = 512
H = 8
D = 128
HD = H * D
P = 128
KC = F // P  # 4 contraction chunks for the projections
INV_SQRT_D = 1.0 / math.sqrt(float(D))
W8SCALE = 64.0  # host pre-scale for fp8 Q/K weights (dodges e4m3 subnormals)

Identity = mybir.ActivationFunctionType.Identity
Exp = mybir.ActivationFunctionType.Exp
DoubleRow = mybir.MatmulPerfMode.DoubleRow


def build_nc():
    nc = bacc.Bacc("TRN2", target_bir_lowering=False, debug=False)

    xT_d = nc.dram_tensor("xT", [B_PER, P, KC, C], BF16, kind="ExternalInput")
    xT8_d = nc.dram_tensor("xT8", [B_PER, P, KC, C], FP8, kind="ExternalInput")
    wq8_d = nc.dram_tensor("wq8", [P, KC, HD], FP8, kind="ExternalInput")
    wk8_d = nc.dram_tensor("wk8", [P, KC, HD], FP8, kind="ExternalInput")
    wv_d = nc.dram_tensor("wvc", [P, KC, HD], BF16, kind="ExternalInput")
    w0_d = nc.dram_tensor("w0c", [P, H, D], BF16, kind="ExternalInput")
    bqs_d = nc.dram_tensor("bqs", [P, H], F32, kind="ExternalInput")
    bks_d = nc.dram_tensor("bks", [P, H], F32, kind="ExternalInput")
    bv_d = nc.dram_tensor("bvh", [HD], BF16, kind="ExternalInput")
    bw0_d = nc.dram_tensor("bw0", [D], F32, kind="ExternalInput")
    out_d = nc.dram_tensor("out", [B_PER, C, D], BF16, kind="ExternalOutput")

    with tile.TileContext(nc) as tc:
        with (
            tc.tile_pool(name="const", bufs=1) as constp,
            tc.tile_pool(name="wts", bufs=1) as wtsp,
            tc.tile_pool(name="xt", bufs=2) as xtp,
            tc.tile_pool(name="projqk", bufs=2) as projp,
            tc.tile_pool(name="projv", bufs=1) as pvp,
            tc.tile_pool(name="z", bufs=2) as zp,
            tc.tile_pool(name="tree", bufs=1) as treep,
            tc.tile_pool(name="att", bufs=2) as attp,
            tc.tile_pool(name="outp", bufs=2) as outp,
            tc.tile_pool(name="ps2", bufs=2, space="PSUM") as ps2p,   # 2-bank tiles
            tc.tile_pool(name="ps1", bufs=2, space="PSUM") as ps1p,   # 1-bank tiles
            tc.tile_pool(name="psout", bufs=1, space="PSUM") as psoutp,  # 2 banks
        ):
            # ---- weights (host-staged layouts, straight DMA) ----
            wq8 = wtsp.tile([P, KC, HD], FP8, name="wq8")
            nc.sync.dma_start(wq8[:], wq8_d[:])
            wk8 = wtsp.tile([P, KC, HD], FP8, name="wk8")
            nc.scalar.dma_start(wk8[:], wk8_d[:])
            wv = wtsp.tile([P, KC, HD], BF16, name="wv")
            nc.scalar.dma_start(wv[:], wv_d[:])
            w0sb = constp.tile([P, H, D], BF16, name="w0sb")
            nc.scalar.dma_start(w0sb[:], w0_d[:])

            # ---- constants ----
            ident = constp.tile([P, P], F32, name="ident")
            make_identity(nc, ident)
            ident4 = constp.tile([P, 4, P], BF16, name="ident4")
            for k in range(4):
                nc.vector.tensor_copy(out=ident4[:, k, :], in_=ident[:])
            ones = constp.tile([P, P], BF16, name="ones")
            nc.gpsimd.memset(ones, 1.0)

            bqss = constp.tile([P, H], F32, name="bqss")
            nc.sync.dma_start(bqss[:], bqs_d[:])
            bksb = constp.tile([P, H], F32, name="bksb")
            nc.sync.dma_start(bksb[:], bks_d[:])
            bw0sb = constp.tile([P, 1], F32, name="bw0sb")
            nc.sync.dma_start(bw0sb[:], bw0_d[:, None])
            bvb = constp.tile([P, HD], BF16, name="bvb")
            nc.gpsimd.dma_start(bvb[:], bv_d[None, :].to_broadcast([P, HD]))

            for b in range(B_PER):
                # ---- x^T tiles: straight per-chunk DMAs ----
                xT8 = xtp.tile([P, KC, C], FP8, name="xT8", tag="xT8")
                for k in range(KC):
                    nc.sync.dma_start(xT8[:, k, :], xT8_d[b, :, k, :])
                xT = xtp.tile([P, KC, C], BF16, name="xT", tag="xT")
                for k in range(KC):
                    nc.scalar.dma_start(xT[:, k, :], xT_d[b, :, k, :])

                # ---- projections ----
                pqT = projp.tile([P, H, C], BF16, name="pqT", tag="pq")
                pkT = projp.tile([P, H, C], BF16, name="pkT", tag="pk")
                pv = pvp.tile([P, C // P, HD], BF16, name="pv", tag="pv")

                # fp8 DoubleRow Q^T/K^T: out[hd-chunk t, c] = W^T x^T (+bias)
                for t in range(H):
                    for w8, bias, scale, dst in (
                        (wq8, bqss, INV_SQRT_D / W8SCALE, pqT),
                        (wk8, bksb, 1.0 / W8SCALE, pkT),
                    ):
                        ps = ps2p.tile([P, C], F32, name="ps_qk", tag="ps2")
                        for s in range(2):
                            for a in range(2):
                                nc.tensor.matmul(
                                    ps[:, 512 * s : 512 * (s + 1)],
                                    lhsT=w8[:, 2 * a : 2 * a + 2, P * t : P * (t + 1)],
                                    rhs=xT8[:, 2 * a : 2 * a + 2, 512 * s : 512 * (s + 1)],
                                    start=(a == 0),
                                    stop=(a == 1),
                                    perf_mode=DoubleRow,
                                )
                        nc.scalar.activation(
                            dst[:, t, :],
                            ps[:],
                            Identity,
                            bias=bias[:, t : t + 1],
                            scale=scale,
                        )

                # natural V (bf16): out[c-chunk j, hd] = x Wv + bv
                for j in range(C // P):
                    ps = ps2p.tile([P, C], F32, name="ps_v", tag="ps2")
                    for s in range(2):
                        for k in range(KC):
                            nc.tensor.matmul(
                                ps[:, 512 * s : 512 * (s + 1)],
                                lhsT=xT[:, k, P * j : P * (j + 1)],
                                rhs=wv[:, k, 512 * s : 512 * (s + 1)],
                                start=(k == 0),
                                stop=(k == KC - 1),
                            )
                    for s in range(2):
                        sl = slice(512 * s, 512 * (s + 1))
                        nc.vector.tensor_add(
                            out=pv[:, j, sl], in0=ps[:, sl], in1=bvb[:, sl]
                        )

                # ---- attention over 8 groups, software pipelined ----
                outacc = psoutp.tile([P, C], F32, name="outacc", tag="outacc")
                zs = {}

                def emit_scores(g):
                    # High priority: the exp stream paces attention; scores
                    # pairs must outrank the previous groups' tail matmuls in
                    # the PE queue or ACT starves.
                    with tc.high_priority(offset=220):
                        z = zp.tile([P, H, C], BF16, name="z", tag="z")
                        zs[g] = z
                        for h2 in range(H):
                            ps = ps2p.tile([P, C], F32, name="ps_s", tag="ps2")
                            for s in range(2):
                                nc.tensor.matmul(
                                    ps[:, 512 * s : 512 * (s + 1)],
                                    lhsT=pkT[:, h2, P * g : P * (g + 1)],
                                    rhs=pqT[:, 4 * s : 4 * (s + 1), P * g : P * (g + 1)],
                                    start=True,
                                    stop=True,
                                )
                            nc.scalar.activation(z[:, h2, :], ps[:], Exp)

                def emit_tail(g):
                    z = zs.pop(g)
                    zs4 = treep.tile([P, 4, C], BF16, name="zs4", tag="zs4")
                    nc.vector.tensor_add(out=zs4[:], in0=z[:, 0:4, :], in1=z[:, 4:8, :])
                    zs2 = treep.tile([P, 2, C], BF16, name="zs2", tag="zs2")
                    nc.vector.tensor_add(
                        out=zs2[:], in0=zs4[:, 0:2, :], in1=zs4[:, 2:4, :]
                    )

                    vals = attp.tile([P, C], BF16, name="vals", tag="vals")
                    rcps = []
                    for s in range(2):
                        pr = ps1p.tile([P, 512], F32, name="pr", tag="ps1")
                        for a in range(2):
                            nc.tensor.matmul(
                                pr[:],
                                lhsT=ones[:],
                                rhs=zs2[:, a, 512 * s : 512 * (s + 1)],
                                start=(a == 0),
                                stop=(a == 1),
                            )
                        rcp = attp.tile([P, 512], F32, name="rcp", tag="rcp")
                        rcps.append(rcp)
                        nc.vector.reciprocal_approx_fast(rcp[:], pr[:])
                        dgm = attp.tile([P, 4, P], BF16, name="dgm", tag="dgm")
                        nc.vector.tensor_mul(
                            out=dgm[:],
                            in0=ident4[:],
                            in1=pr[:].rearrange("p (a j) -> p a j", j=P),
                        )
                        for h2 in range(4 * s, 4 * (s + 1)):
                            nc.gpsimd.tensor_sub(
                                out=z[:, h2, P * h2 : P * (h2 + 1)],
                                in0=z[:, h2, P * h2 : P * (h2 + 1)],
                                in1=dgm[:, h2 - 4 * s, :],
                            )

                    for s in range(2):
                        pvz = ps1p.tile([P, 512], F32, name="pvz", tag="ps1")
                        for h2 in range(H):
                            nc.tensor.matmul(
                                pvz[:],
                                lhsT=pv[:, g, P * h2 : P * (h2 + 1)],
                                rhs=z[:, h2, 512 * s : 512 * (s + 1)],
                                start=(h2 == 0),
                                stop=(h2 == H - 1),
                            )
                        nc.vector.tensor_mul(
                            out=vals[:, 512 * s : 512 * (s + 1)],
                            in0=pvz[:],
                            in1=rcps[s][:],
                        )

                    for s in range(2):
                        nc.tensor.matmul(
                            outacc[:, 512 * s : 512 * (s + 1)],
                            lhsT=w0sb[:, g, :],
                            rhs=vals[:, 512 * s : 512 * (s + 1)],
                            start=(g == 0),
                            stop=(g == H - 1),
                        )

                for g in range(H):
                    emit_scores(g)
                    if g > 0:
                        emit_tail(g - 1)
                emit_tail(H - 1)

                # ---- drain + un-permute: xbar transpose + plain store ----
                outTb = outp.tile([P, C], BF16, name="outTb", tag="outTb")
                nc.scalar.activation(
                    outTb[:], outacc[:], Identity, bias=bw0sb[:, 0:1]
                )
                outTT = outp.tile([P, H, D], BF16, name="outTT", tag="outTT")
                nc.sync.dma_start_transpose(outTT[:], outTb[:])
                nc.sync.dma_start(
                    out_d[b].rearrange("(cm e) d -> cm e d", e=H), outTT[:]
                )

    return nc


_NC_CACHE = None


def _get_nc():
    global _NC_CACHE
    if _NC_CACHE is None:
        nc = build_nc()
        nc.compile()  # Bacc passes: move matmul waits to ldweights, alloc regs
        _NC_CACHE = nc
    return _NC_CACHE


def _install_ntff_shim():
    """The agent image's antenv lacks axon_hooks, so trn_boot's NTFF hook
    registration silently degrades. Recreate the module and register the
    ctypes-based hook so trace=True produces a profile."""
    import sys
    import types

    try:
        import antenv  # noqa: F401
        from antenv import axon_hooks  # noqa: F401

        return  # already present
    except ImportError:
        pass
    mod = types.ModuleType("antenv.axon_hooks")
    _state = {"hook": None}
    mod.set_axon_ntff_profile_hook = lambda h: _state.__setitem__("hook", h)
    mod.get_axon_ntff_profile_hook = lambda: _state["hook"]
    sys.modules["antenv.axon_hooks"] = mod
    import antenv

    antenv.axon_hooks = mod
    try:
        from trn_agent_boot.trn_boot import _ntff_profile_via_ctypes

        hook = _ntff_profile_via_ctypes("/opt/axon/libaxon_pjrt.so")
        if hook is not None:
            mod.set_axon_ntff_profile_hook(hook)
    except Exception as e:  # degrade to no tracing
        print(f"ntff shim failed: {e}")


def _host_stage(inputs):
    """Cast/layout all operands on the host so the device DMAs bf16/fp8."""
    f32 = np.float32
    bf16 = ml_dtypes.bfloat16
    fp8 = ml_dtypes.float8_e4m3fn

    Wq = np.asarray(inputs["Wq"], f32)
    Wk = np.asarray(inputs["Wk"], f32)
    Wv = np.asarray(inputs["Wv"], f32)

    def chunk(w):  # [F, HD] -> [P, KC, HD]  (w[128k+p, hd] -> [p, k, hd])
        return np.ascontiguousarray(w.reshape(KC, P, HD).transpose(1, 0, 2))

    weights = {
        "wq8": (chunk(Wq) * W8SCALE).astype(fp8),
        "wk8": (chunk(Wk) * W8SCALE).astype(fp8),
        "wvc": chunk(Wv).astype(bf16),
        "w0c": np.ascontiguousarray(
            np.asarray(inputs["Ww0"], f32).reshape(H, P, D).transpose(1, 0, 2)
        ).astype(bf16),
        "bqs": np.ascontiguousarray(
            (np.asarray(inputs["bq"], f32) * INV_SQRT_D).reshape(H, P).T
        ),
        "bks": np.ascontiguousarray(np.asarray(inputs["bk"], f32).reshape(H, P).T),
        "bvh": np.asarray(inputs["bv"], f32).astype(bf16),
        "bw0": np.asarray(inputs["bw0"], f32),
    }

    x = np.asarray(inputs["x"], f32)  # [B, C, F]
    # xT[b, p, k, c] = x[b, c, 128k + p]
    xT = np.ascontiguousarray(
        x.transpose(0, 2, 1).reshape(x.shape[0], KC, P, C).transpose(0, 2, 1, 3)
    )
    return weights, xT.astype(bf16), xT.astype(fp8)


def kernel_with_results(trace=False, **inputs):
    if trace:
        _install_ntff_shim()
    nc = _get_nc()
    weights, xT, xT8 = _host_stage(inputs)
    in_maps = []
    for i in range(N_CORES):
        m = {
            "xT": np.ascontiguousarray(xT[B_PER * i : B_PER * (i + 1)]),
            "xT8": np.ascontiguousarray(xT8[B_PER * i : B_PER * (i + 1)]),
        }
        m.update(weights)
        in_maps.append(m)
    res = run_bass_kernel_spmd(nc, in_maps, list(range(N_CORES)), trace=trace)
    out = np.concatenate(
        [res.results[i]["out"].astype(np.float32) for i in range(N_CORES)], axis=0
    )
    return out, res


def kernel(**inputs):
    out, _ = kernel_with_results(trace=False, **inputs)
    return out


# revision 22
# speedup vs baseline: 1.0031x; 1.0031x over previous
"""Trainium2 Bass kernel for the GRAND attention block.

Shapes (hardcoded): B=16, C=1024, F=512, H=8, D=128, HD=1024.
Sharding: batch dim split across 8 cores (2 batches per core), weights
replicated; no collectives needed.

Math per batch (b):
  P_q = (x Wq + bq)/sqrt(D), P_k = x Wk + bk, P_v = x Wv + bv  [1024, 1024]
  The reference reshape [C, H*D] -> [H, C, D] (no permute) makes
  "head" g = proj_row // 128 and attention row c'' = 128*e + m where
  e = colblock, m = proj_row % 128.  Attention runs independently inside
  each group g of 1024 rows.

    S^T tiles = K_e2 Q_e1^T  -> exp -> Z = E^T            [1024, 1024]
    r = column sums of Z (DVE tree + accumulating ones-matmuls)
    Z' = Z - diag(r)   (handles softmax denominator and the "- I" term)
    vals^T = (V_g^T Z') / r
    out^T += W0_g^T vals^T  (PSUM-accumulated over g), + bw0
  out^T is DMA-xbar transposed and stored with the unscrambling view.

Perf design: all dtype casts and weight/x layout transforms happen on the
HOST (numpy) so the device sees bf16/fp8 tensors it can DMA directly —
the fp32 load path was DMA-bandwidth-bound (~80us startup).  Q/K
projections run as fp8 DoubleRow matmuls (K folded 512 -> 2x256, weights
host-prescaled by 64 to dodge e4m3 subnormals; 1/64 and 1/sqrt(D) fold
into the drain's activation scale).  Attention is software-pipelined:
scores+exp of group g are emitted before the tail of g-1 so the ACT exp
stream never starves; column sums use a DVE half-reduction plus
accumulating ones-matmuls; the diag subtraction runs on GpSimd; vals are
scaled by reciprocal_approx_fast.  PSUM: 2x two-bank score tiles, 2x
one-bank r/vals tiles, one two-bank out accumulator.
"""

import math

import numpy as np
import ml_dtypes

import concourse.bass as bass
import concourse.bacc as bacc
import concourse.mybir as mybir
import concourse.tile as tile
from concourse.masks import make_identity
from concourse.bass_utils import run_bass_kernel_spmd

F32 = mybir.dt.float32
BF16 = mybir.dt.bfloat16
FP8 = mybir.dt.float8e4

N_CORES = 8
B_PER = 2  # batches per core
C = 1024
F = 512
H = 8
D = 128
HD = H * D
P = 128
KC = F // P  # 4 contraction chunks for the projections
INV_SQRT_D = 1.0 / math.sqrt(float(D))
W8SCALE = 64.0  # host pre-scale for fp8 Q/K weights (dodges e4m3 subnormals)

Identity = mybir.ActivationFunctionType.Identity
Exp = mybir.ActivationFunctionType.Exp
DoubleRow = mybir.MatmulPerfMode.DoubleRow


def build_nc():
    nc = bacc.Bacc("TRN2", target_bir_lowering=False, debug=False)

    xT_d = nc.dram_tensor("xT", [B_PER, P, KC, C], BF16, kind="ExternalInput")
    xT8_d = nc.dram_tensor("xT8", [B_PER, P, KC, C], FP8, kind="ExternalInput")
    wq8_d = nc.dram_tensor("wq8", [P, KC, HD], FP8, kind="ExternalInput")
    wk8_d = nc.dram_tensor("wk8", [P, KC, HD], FP8, kind="ExternalInput")
    wv_d = nc.dram_tensor("wvc", [P, KC, HD], BF16, kind="ExternalInput")
    w0_d = nc.dram_tensor("w0c", [P, H, D], BF16, kind="ExternalInput")
    bqs_d = nc.dram_tensor("bqs", [P, H], F32, kind="ExternalInput")
    bks_d = nc.dram_tensor("bks", [P, H], F32, kind="ExternalInput")
    bv_d = nc.dram_tensor("bvh", [HD], BF16, kind="ExternalInput")
    bw0_d = nc.dram_tensor("bw0", [D], F32, kind="ExternalInput")
    out_d = nc.dram_tensor("out", [B_PER, C, D], BF16, kind="ExternalOutput")

    with tile.TileContext(nc) as tc:
        with (
            tc.tile_pool(name="const", bufs=1) as constp,
            tc.tile_pool(name="wts", bufs=1) as wtsp,
            tc.tile_pool(name="xt", bufs=2) as xtp,
            tc.tile_pool(name="projqk", bufs=2) as projp,
            tc.tile_pool(name="projv", bufs=1) as pvp,
            tc.tile_pool(name="z", bufs=2) as zp,
            tc.tile_pool(name="tree", bufs=1) as treep,
            tc.tile_pool(name="att", bufs=2) as attp,
            tc.tile_pool(name="outp", bufs=2) as outp,
            tc.tile_pool(name="ps2", bufs=2, space="PSUM") as ps2p,   # 2-bank tiles
            tc.tile_pool(name="ps1", bufs=2, space="PSUM") as ps1p,   # 1-bank tiles
            tc.tile_pool(name="psout", bufs=1, space="PSUM") as psoutp,  # 2 banks
        ):
            # ---- weights (host-staged layouts, straight DMA) ----
            # order: wq8 then xT8 (emitted in the b loop) gate the first
            # matmul chain; wk8/wv/w0 stream behind
            wq8 = wtsp.tile([P, KC, HD], FP8, name="wq8")
            nc.sync.dma_start(wq8[:], wq8_d[:])
            wk8 = wtsp.tile([P, KC, HD], FP8, name="wk8")
            nc.scalar.dma_start(wk8[:], wk8_d[:])
            wv = wtsp.tile([P, KC, HD], BF16, name="wv")
            nc.scalar.dma_start(wv[:], wv_d[:])
            w0sb = constp.tile([P, H, D], BF16, name="w0sb")
            nc.scalar.dma_start(w0sb[:], w0_d[:])
            kdrain_scale = 1.0 / W8SCALE

            # ---- constants ----
            ident = constp.tile([P, P], F32, name="ident")
            make_identity(nc, ident)
            ident4 = constp.tile([P, 4, P], BF16, name="ident4")
            for k in range(4):
                nc.vector.tensor_copy(out=ident4[:, k, :], in_=ident[:])
            ones = constp.tile([P, P], BF16, name="ones")
            nc.gpsimd.memset(ones, 1.0)

            bqss = constp.tile([P, H], F32, name="bqss")
            nc.sync.dma_start(bqss[:], bqs_d[:])
            bksb = constp.tile([P, H], F32, name="bksb")
            nc.sync.dma_start(bksb[:], bks_d[:])
            bw0sb = constp.tile([P, 1], F32, name="bw0sb")
            nc.sync.dma_start(bw0sb[:], bw0_d[:, None])
            bvb = constp.tile([P, HD], BF16, name="bvb")
            nc.gpsimd.dma_start(bvb[:], bv_d[None, :].to_broadcast([P, HD]))

            for b in range(B_PER):
                # ---- x^T tiles: straight per-chunk DMAs ----
                xT8 = xtp.tile([P, KC, C], FP8, name="xT8", tag="xT8")
                for k in range(KC):
                    nc.sync.dma_start(xT8[:, k, :], xT8_d[b, :, k, :])
                xT = xtp.tile([P, KC, C], BF16, name="xT", tag="xT")
                for k in range(KC):
                    nc.scalar.dma_start(xT[:, k, :], xT_d[b, :, k, :])

                # ---- projections ----
                pqT = projp.tile([P, H, C], BF16, name="pqT", tag="pq")
                pkT = projp.tile([P, H, C], BF16, name="pkT", tag="pk")
                pv = pvp.tile([P, C // P, HD], BF16, name="pv", tag="pv")

                # fp8 DoubleRow Q^T/K^T: out[hd-chunk t, c] = W^T x^T (+bias)
                # Q drains on ACT (scale+bias); K drains on DVE halves
                # (host pre-scales bk by 64 so (ps + 64 bk)/64 works fused)
                for t in range(H):
                    for w8, dst in ((wq8, pqT), (wk8, pkT)):
                        ps = ps2p.tile([P, C], F32, name="ps_qk", tag="ps2")
                        for s in range(2):
                            for a in range(2):
                                nc.tensor.matmul(
                                    ps[:, 512 * s : 512 * (s + 1)],
                                    lhsT=w8[:, 2 * a : 2 * a + 2, P * t : P * (t + 1)],
                                    rhs=xT8[:, 2 * a : 2 * a + 2, 512 * s : 512 * (s + 1)],
                                    start=(a == 0),
                                    stop=(a == 1),
                                    perf_mode=DoubleRow,
                                )
                        if dst is pqT:
                            nc.scalar.activation(
                                dst[:, t, :],
                                ps[:],
                                Identity,
                                bias=bqss[:, t : t + 1],
                                scale=INV_SQRT_D / W8SCALE,
                            )
                        else:
                            for s in range(2):
                                sl = slice(512 * s, 512 * (s + 1))
                                nc.vector.tensor_scalar(
                                    out=dst[:, t, sl],
                                    in0=ps[:, sl],
                                    scalar1=bksb[:, t : t + 1],
                                    scalar2=kdrain_scale,
                                    op0=mybir.AluOpType.add,
                                    op1=mybir.AluOpType.mult,
                                )

                # natural V (bf16): out[c-chunk j, hd] = x Wv + bv
                for j in range(C // P):
                    ps = ps2p.tile([P, C], F32, name="ps_v", tag="ps2")
                    for s in range(2):
                        for k in range(KC):
                            nc.tensor.matmul(
                                ps[:, 512 * s : 512 * (s + 1)],
                                lhsT=xT[:, k, P * j : P * (j + 1)],
                                rhs=wv[:, k, 512 * s : 512 * (s + 1)],
                                start=(k == 0),
                                stop=(k == KC - 1),
                            )
                    for s in range(2):
                        sl = slice(512 * s, 512 * (s + 1))
                        nc.vector.tensor_add(
                            out=pv[:, j, sl], in0=ps[:, sl], in1=bvb[:, sl]
                        )

                # ---- attention over 8 groups, software pipelined ----
                outacc = psoutp.tile([P, C], F32, name="outacc", tag="outacc")
                zs = {}

                def emit_scores(g):
                    z = zp.tile([P, H, C], BF16, name="z", tag="z")
                    zs[g] = z
                    for h2 in range(H):
                        ps = ps2p.tile([P, C], F32, name="ps_s", tag="ps2")
                        for s in range(2):
                            nc.tensor.matmul(
                                ps[:, 512 * s : 512 * (s + 1)],
                                lhsT=pkT[:, h2, P * g : P * (g + 1)],
                                rhs=pqT[:, 4 * s : 4 * (s + 1), P * g : P * (g + 1)],
                                start=True,
                                stop=True,
                            )
                        nc.scalar.activation(z[:, h2, :], ps[:], Exp)

                def emit_tail(g):
                    z = zs.pop(g)
                    zs4 = treep.tile([P, 4, C], BF16, name="zs4", tag="zs4")
                    nc.vector.tensor_add(out=zs4[:], in0=z[:, 0:4, :], in1=z[:, 4:8, :])
                    zs2 = treep.tile([P, 2, C], BF16, name="zs2", tag="zs2")
                    nc.vector.tensor_add(
                        out=zs2[:], in0=zs4[:, 0:2, :], in1=zs4[:, 2:4, :]
                    )

                    vals = attp.tile([P, C], BF16, name="vals", tag="vals")
                    rcps = []
                    for s in range(2):
                        pr = ps1p.tile([P, 512], F32, name="pr", tag="ps1")
                        for a in range(2):
                            nc.tensor.matmul(
                                pr[:],
                                lhsT=ones[:],
                                rhs=zs2[:, a, 512 * s : 512 * (s + 1)],
                                start=(a == 0),
                                stop=(a == 1),
                            )
                        rcp = attp.tile([P, 512], F32, name="rcp", tag="rcp")
                        rcps.append(rcp)
                        nc.vector.reciprocal_approx_fast(rcp[:], pr[:])
                        dgm = attp.tile([P, 4, P], BF16, name="dgm", tag="dgm")
                        nc.vector.tensor_mul(
                            out=dgm[:],
                            in0=ident4[:],
                            in1=pr[:].rearrange("p (a j) -> p a j", j=P),
                        )
                        for h2 in range(4 * s, 4 * (s + 1)):
                            nc.gpsimd.tensor_sub(
                                out=z[:, h2, P * h2 : P * (h2 + 1)],
                                in0=z[:, h2, P * h2 : P * (h2 + 1)],
                                in1=dgm[:, h2 - 4 * s, :],
                            )

                    for s in range(2):
                        pvz = ps1p.tile([P, 512], F32, name="pvz", tag="ps1")
                        for h2 in range(H):
                            nc.tensor.matmul(
                                pvz[:],
                                lhsT=pv[:, g, P * h2 : P * (h2 + 1)],
                                rhs=z[:, h2, 512 * s : 512 * (s + 1)],
                                start=(h2 == 0),
                                stop=(h2 == H - 1),
                            )
                        nc.vector.tensor_mul(
                            out=vals[:, 512 * s : 512 * (s + 1)],
                            in0=pvz[:],
                            in1=rcps[s][:],
                        )

                    for s in range(2):
                        nc.tensor.matmul(
                            outacc[:, 512 * s : 512 * (s + 1)],
                            lhsT=w0sb[:, g, :],
                            rhs=vals[:, 512 * s : 512 * (s + 1)],
                            start=(g == 0),
                            stop=(g == H - 1),
                        )

                for g in range(H):
                    emit_scores(g)
                    if g > 0:
                        emit_tail(g - 1)
                emit_tail(H - 1)

                # ---- drain + un-permute: xbar transpose + plain store ----
                outTb = outp.tile([P, C], BF16, name="outTb", tag="outTb")
                nc.scalar.activation(
                    outTb[:], outacc[:], Identity, bias=bw0sb[:, 0:1]
                )
                outTT = outp.tile([P, H, D], BF16, name="outTT", tag="outTT")
                nc.sync.dma_start_transpose(outTT[:], outTb[:])
                nc.sync.dma_start(
                    out_d[b].rearrange("(cm e) d -> cm e d", e=H), outTT[:]
                )

    return nc


_NC_CACHE = None


def _get_nc():
    global _NC_CACHE
    if _NC_CACHE is None:
        nc = build_nc()
        nc.compile()  # Bacc passes: move matmul waits to ldweights, alloc regs
        _NC_CACHE = nc
    return _NC_CACHE


def _install_ntff_shim():
    """The agent image's antenv lacks axon_hooks, so trn_boot's NTFF hook
    registration silently degrades. Recreate the module and register the
    ctypes-based hook so trace=True produces a profile."""
    import sys
    import types

    try:
        import antenv  # noqa: F401
        from antenv import axon_hooks  # noqa: F401

        return  # already present
    except ImportError:
        pass
    mod = types.ModuleType("antenv.axon_hooks")
    _state = {"hook": None}
    mod.set_axon_ntff_profile_hook = lambda h: _state.__setitem__("hook", h)
    mod.get_axon_ntff_profile_hook = lambda: _state["hook"]
    sys.modules["antenv.axon_hooks"] = mod
    import antenv

    antenv.axon_hooks = mod
    try:
        from trn_agent_boot.trn_boot import _ntff_profile_via_ctypes

        hook = _ntff_profile_via_ctypes("/opt/axon/libaxon_pjrt.so")
        if hook is not None:
            mod.set_axon_ntff_profile_hook(hook)
    except Exception as e:  # degrade to no tracing
        print(f"ntff shim failed: {e}")


def _host_stage(inputs):
    """Cast/layout all operands on the host so the device DMAs bf16/fp8."""
    f32 = np.float32
    bf16 = ml_dtypes.bfloat16
    fp8 = ml_dtypes.float8_e4m3fn

    Wq = np.asarray(inputs["Wq"], f32)
    Wk = np.asarray(inputs["Wk"], f32)
    Wv = np.asarray(inputs["Wv"], f32)

    def chunk(w):  # [F, HD] -> [P, KC, HD]  (w[128k+p, hd] -> [p, k, hd])
        return np.ascontiguousarray(w.reshape(KC, P, HD).transpose(1, 0, 2))

    weights = {
        "wq8": (chunk(Wq) * W8SCALE).astype(fp8),
        "wk8": (chunk(Wk) * W8SCALE).astype(fp8),
        "wvc": chunk(Wv).astype(bf16),
        "w0c": np.ascontiguousarray(
            np.asarray(inputs["Ww0"], f32).reshape(H, P, D).transpose(1, 0, 2)
        ).astype(bf16),
        "bqs": np.ascontiguousarray(
            (np.asarray(inputs["bq"], f32) * INV_SQRT_D).reshape(H, P).T
        ),
        "bks": np.ascontiguousarray(
            (np.asarray(inputs["bk"], f32) * W8SCALE).reshape(H, P).T
        ),
        "bvh": np.asarray(inputs["bv"], f32).astype(bf16),
        "bw0": np.asarray(inputs["bw0"], f32),
    }

    x = np.asarray(inputs["x"], f32)  # [B, C, F]
    # xT[b, p, k, c] = x[b, c, 128k + p]
    xT = np.ascontiguousarray(
        x.transpose(0, 2, 1).reshape(x.shape[0], KC, P, C).transpose(0, 2, 1, 3)
    )
    return weights, xT.astype(bf16), xT.astype(fp8)


def kernel_with_results(trace=False, **inputs):
    if trace:
        _install_ntff_shim()
    nc = _get_nc()
    weights, xT, xT8 = _host_stage(inputs)
    in_maps = []
    for i in range(N_CORES):
        m = {
            "xT": np.ascontiguousarray(xT[B_PER * i : B_PER * (i + 1)]),
            "xT8": np.ascontiguousarray(xT8[B_PER * i : B_PER * (i + 1)]),
        }
        m.update(weights)
        in_maps.append(m)
    res = run_bass_kernel_spmd(nc, in_maps, list(range(N_CORES)), trace=trace)
    out = np.concatenate(
        [res.results[i]["out"].astype(np.float32) for i in range(N_CORES)], axis=0
    )
    return out, res


def kernel(**inputs):
    out, _ = kernel_with_results(trace=False, **inputs)
    return out
